# revision 7
# baseline (speedup 1.0000x reference)
"""Trainium2 Bass/Tile kernel for AttentionCombinerWithResidual.

Reference computation (per batch element b):
    q = x_t @ Wq.T + bq ; k = x_s @ Wk.T + bk ; v = x_s @ Wv.T + bv
    w = softmax(q @ k.T / sqrt(D))
    out = layernorm(w @ v + x_t) * gamma + beta

Sharding: data-parallel over batch B=8 -> 8 NeuronCores, weights replicated,
no cross-core communication. Each core runs the full S=4096, D=256 attention.

Per-core device algorithm:
  - load x_t/x_s [4096,256] f32, PE-transpose into x_T [d,s] bf16
  - project qT/kT [d_out, s] and v [s, d_out] (bf16, f32 PSUM accum),
    biases fused into the PSUM->SBUF copies; v gets a ones column appended so
    the attention row-sums fall out of the combined matmul for free
  - per 1024-query chunk: scoresT[j,i] matmuls -> ACT exp(x/16) (safe without
    max subtraction: scores ~ N(0,1) after scaling) -> bf16 w tiles ->
    combined[i, 257] accumulated over j (column 256 = softmax denominator)
  - epilogue per 128-row block: scale by 1/denominator, residual add,
    bn_stats/bn_aggr layernorm stats, Newton-iteration rsqrt on the vector
    engine (avoids switching the scalar engine's activation table away from
    Exp), gamma/beta on gpsimd, DMA out f32
"""

import numpy as np
import ml_dtypes

B = 8
S = 4096
D = 256
P = 128
ND = D // P        # 2 d-tiles
NS = S // P        # 32 s-tiles
CH = 1024          # query-chunk width
NCH = S // CH      # 4 chunks
NQ = CH // 512     # 512-wide matmul slices per scores psum tile
NIB = CH // P      # 8 i-blocks per chunk
EPS = 1e-5
SCALE = 1.0 / 16.0  # 1/sqrt(D)

_CACHE = {}


def _build_nc(repeat=1, loop_n=0):
    """loop_n > 0 wraps the body in a device-side For_i loop (timing builds:
    the NEFF size stays constant while hardware work scales with loop_n)."""
    import contextlib

    import concourse.bacc as bacc
    import concourse.bass as bass
    import concourse.tile as tile
    import concourse.mybir as mybir

    f32 = mybir.dt.float32
    bf16 = mybir.dt.bfloat16
    u32 = mybir.dt.uint32
    i32 = mybir.dt.int32
    AF = mybir.ActivationFunctionType
    OP = mybir.AluOpType

    nc = bacc.Bacc("TRN2", target_bir_lowering=False, debug=False)

    xt_d = nc.dram_tensor("xt", [S, D], f32, kind="ExternalInput")
    xs_d = nc.dram_tensor("xs", [S, D], f32, kind="ExternalInput")
    wq_d = nc.dram_tensor("wqt", [D, D], bf16, kind="ExternalInput")  # Wq.T = [d, d_out]
    wk_d = nc.dram_tensor("wkt", [D, D], bf16, kind="ExternalInput")
    wv_d = nc.dram_tensor("wvt", [D, D], bf16, kind="ExternalInput")
    bq_d = nc.dram_tensor("bq", [D], f32, kind="ExternalInput")
    bk_d = nc.dram_tensor("bk", [D], f32, kind="ExternalInput")
    bv_d = nc.dram_tensor("bv", [D], f32, kind="ExternalInput")
    g_d = nc.dram_tensor("gamma", [D], f32, kind="ExternalInput")
    be_d = nc.dram_tensor("beta", [D], f32, kind="ExternalInput")
    out_d = nc.dram_tensor("out", [S, D], f32, kind="ExternalOutput")

    def bcast(dram_ap, n):
        # [free...] dram vector -> [n, free...] stride-0 partition broadcast
        return bass.AP(
            tensor=dram_ap.tensor, offset=dram_ap.offset, ap=[[0, n]] + list(dram_ap.ap)
        )

    xt_rows = xt_d.ap().rearrange("(t p) c -> t p c", p=P)
    xs_rows = xs_d.ap().rearrange("(t p) c -> t p c", p=P)
    out_rows = out_d.ap().rearrange("(t p) c -> t p c", p=P)

    with tile.TileContext(nc) as tc:
        with (
            tc.tile_pool(name="persist", bufs=1) as persist,
            tc.tile_pool(name="xload", bufs=4) as xload,
            tc.tile_pool(name="wpool", bufs=36) as wpool,
            tc.tile_pool(name="epi", bufs=4) as epi,
            tc.tile_pool(name="stats", bufs=8) as stats,
            tc.tile_pool(name="psA", bufs=3, space="PSUM") as psA,
            tc.tile_pool(name="psB", bufs=2, space="PSUM") as psB,
        ):
            from concourse.masks import make_identity

            ident = persist.tile([P, P], f32)
            make_identity(nc, ident)

            # replicated constants
            wqs = persist.tile([P, ND, D], bf16)
            wks = persist.tile([P, ND, D], bf16)
            wvs = persist.tile([P, ND, D], bf16)
            nc.sync.dma_start(wqs[:], wq_d.ap().rearrange("(t p) c -> p t c", p=P))
            nc.sync.dma_start(wks[:], wk_d.ap().rearrange("(t p) c -> p t c", p=P))
            nc.sync.dma_start(wvs[:], wv_d.ap().rearrange("(t p) c -> p t c", p=P))
            bq_sb = persist.tile([P, ND], f32)
            bk_sb = persist.tile([P, ND], f32)
            nc.sync.dma_start(bq_sb[:], bq_d.ap().rearrange("(t p) -> p t", p=P))
            nc.sync.dma_start(bk_sb[:], bk_d.ap().rearrange("(t p) -> p t", p=P))
            bv_bc = persist.tile([P, D], f32)
            gm_bc = persist.tile([P, D], f32)
            bt_bc = persist.tile([P, D], f32)
            nc.sync.dma_start(bv_bc[:], bcast(bv_d.ap(), P))
            nc.sync.dma_start(gm_bc[:], bcast(g_d.ap(), P))
            nc.sync.dma_start(bt_bc[:], bcast(be_d.ap(), P))

            xtT = persist.tile([P, ND, S], bf16)  # [p, kd, s] = x_t[s, kd*P+p]
            xsT = persist.tile([P, ND, S], bf16)
            qT = persist.tile([P, ND, S], bf16)   # [p, mo, s] = q[s, mo*P+p]
            kT = persist.tile([P, ND, S], bf16)
            v_sb = persist.tile([P, NS, D + 1], bf16)  # [p, jt, c]; c==D is ones

            def body():
                # ---- phase 0: load + transpose inputs ----
                for st in range(NS):
                    for src_rows, dstT in ((xt_rows, xtT), (xs_rows, xsT)):
                        xn = xload.tile([P, D], f32, tag="xn")
                        nc.sync.dma_start(xn[:], src_rows[st])
                        for kd in range(ND):
                            pst = psB.tile([P, P], f32, tag="ps_small")
                            nc.tensor.transpose(
                                pst[:], xn[:, kd * P : (kd + 1) * P], ident[:]
                            )
                            nc.vector.tensor_copy(
                                dstT[:, kd, st * P : (st + 1) * P], pst[:]
                            )

                # ---- phase 1: projections ----
                for wsb, xT, bsb, dstT in (
                    (wqs, xtT, bq_sb, qT),
                    (wks, xsT, bk_sb, kT),
                ):
                    for mo in range(ND):
                        for sc in range(S // 512):
                            ps = psB.tile([P, 512], f32, tag="ps_small")
                            for kd in range(ND):
                                nc.tensor.matmul(
                                    ps[:],
                                    wsb[:, kd, mo * P : (mo + 1) * P],
                                    xT[:, kd, sc * 512 : (sc + 1) * 512],
                                    start=(kd == 0),
                                    stop=(kd == ND - 1),
                                )
                            nc.vector.tensor_scalar_add(
                                dstT[:, mo, sc * 512 : (sc + 1) * 512],
                                ps[:],
                                bsb[:, mo : mo + 1],
                            )
                for st in range(NS):
                    ps = psB.tile([P, D], f32, tag="ps_small")
                    for kd in range(ND):
                        nc.tensor.matmul(
                            ps[:],
                            xsT[:, kd, st * P : (st + 1) * P],
                            wvs[:, kd, :],
                            start=(kd == 0),
                            stop=(kd == ND - 1),
                        )
                    nc.vector.tensor_add(v_sb[:, st, 0:D], ps[:], bv_bc[:])
                nc.vector.memset(v_sb[:, :, D : D + 1], 1.0)

                # ---- main loop: attention per query chunk ----
                for c in range(NCH):
                    w_tiles = []
                    for jt in range(NS):
                        ps = psA.tile([P, CH], f32, tag="ps_sc")
                        for q in range(NQ):
                            for kd in range(ND):
                                nc.tensor.matmul(
                                    ps[:, q * 512 : (q + 1) * 512],
                                    kT[:, kd, jt * P : (jt + 1) * P],
                                    qT[
                                        :,
                                        kd,
                                        c * CH + q * 512 : c * CH + (q + 1) * 512,
                                    ],
                                    start=(kd == 0),
                                    stop=(kd == ND - 1),
                                )
                        wt = wpool.tile([P, CH], bf16, tag="w")
                        nc.scalar.activation(wt[:], ps[:], AF.Exp, scale=SCALE)
                        w_tiles.append(wt)

                    for ib in range(NIB):
                        gi = c * NIB + ib
                        pc = psB.tile([P, D + 1], f32, tag="ps_small")
                        for jt in range(NS):
                            nc.tensor.matmul(
                                pc[:],
                                w_tiles[jt][:, ib * P : (ib + 1) * P],
                                v_sb[:, jt, :],
                                start=(jt == 0),
                                stop=(jt == NS - 1),
                            )
                        # epilogue
                        r = stats.tile([P, 1], f32, tag="r")
                        nc.vector.reciprocal(r[:], pc[:, D : D + 1])
                        xn = xload.tile([P, D], f32, tag="xn")
                        nc.sync.dma_start(xn[:], xt_rows[gi])
                        z = epi.tile([P, D], f32, tag="z")
                        nc.vector.tensor_scalar_mul(z[:], pc[:, 0:D], r[:])
                        nc.vector.tensor_add(z[:], z[:], xn[:])
                        st6 = stats.tile([P, 6], f32, tag="st6")
                        nc.vector.bn_stats(st6[:], z[:])
                        mv = stats.tile([P, 2], f32, tag="mv")
                        nc.vector.bn_aggr(mv[:], st6[:])
                        a = stats.tile([P, 1], f32, tag="a")
                        nc.vector.tensor_scalar_add(a[:], mv[:, 1:2], EPS)
                        # y = rsqrt(a): bit-trick seed + 2 Newton iterations
                        y = stats.tile([P, 1], f32, tag="y")
                        yi = y.bitcast(u32)
                        nc.vector.tensor_scalar(
                            yi[:],
                            a.bitcast(u32)[:],
                            1,
                            None,
                            op0=OP.logical_shift_right,
                        )
                        nc.vector.tensor_scalar(
                            yi[:], yi[:], 0xFFFFFFFF, None, op0=OP.bitwise_xor
                        )
                        # K - v == K + (~v), done in SIGNED i32: the uint32 DVE
                        # add saturates at 0xFFFFFFFF instead of wrapping.
                        yi_s = y.bitcast(i32)
                        nc.vector.tensor_scalar(
                            yi_s[:], yi_s[:], 0x5F3759E0, None, op0=OP.add
                        )
                        u = stats.tile([P, 1], f32, tag="u")
                        for _ in range(2):
                            nc.vector.tensor_mul(u[:], y[:], y[:])
                            nc.vector.tensor_mul(u[:], u[:], a[:])
                            nc.vector.tensor_scalar(
                                u[:], u[:], -0.5, 1.5, op0=OP.mult, op1=OP.add
                            )
                            nc.vector.tensor_mul(y[:], y[:], u[:])
                        o = epi.tile([P, D], f32, tag="o")
                        nc.vector.tensor_scalar(
                            o[:],
                            z[:],
                            mv[:, 0:1],
                            y[:],
                            op0=OP.subtract,
                            op1=OP.mult,
                        )
                        nc.gpsimd.tensor_mul(o[:], o[:], gm_bc[:])
                        nc.gpsimd.tensor_add(o[:], o[:], bt_bc[:])
                        nc.sync.dma_start(out_rows[gi], o[:])

            loop_cm = (
                tc.For_i(0, loop_n, 1) if loop_n > 0 else contextlib.nullcontext()
            )
            with loop_cm:
                for _rep in range(repeat):
                    body()

    nc.compile()
    return nc


def _get_nc(repeat=1, loop_n=0):
    key = ("nc", repeat, loop_n)
    if key not in _CACHE:
        _CACHE[key] = _build_nc(repeat, loop_n)
    return _CACHE[key]


def _make_in_maps(
    supervised_embedding,
    transformer_embedding,
    Wq,
    bq,
    Wk,
    bk,
    Wv,
    bv,
    gamma,
    beta,
):
    bf = ml_dtypes.bfloat16
    f32 = np.float32
    shared = {
        "wqt": np.ascontiguousarray(np.asarray(Wq, f32).T).astype(bf),
        "wkt": np.ascontiguousarray(np.asarray(Wk, f32).T).astype(bf),
        "wvt": np.ascontiguousarray(np.asarray(Wv, f32).T).astype(bf),
        "bq": np.ascontiguousarray(np.asarray(bq, f32)),
        "bk": np.ascontiguousarray(np.asarray(bk, f32)),
        "bv": np.ascontiguousarray(np.asarray(bv, f32)),
        "gamma": np.ascontiguousarray(np.asarray(gamma, f32)),
        "beta": np.ascontiguousarray(np.asarray(beta, f32)),
    }
    xs_all = np.asarray(supervised_embedding, f32)
    xt_all = np.asarray(transformer_embedding, f32)
    return [
        {
            "xt": np.ascontiguousarray(xt_all[b]),
            "xs": np.ascontiguousarray(xs_all[b]),
            **shared,
        }
        for b in range(B)
    ]


def kernel(**inputs):
    from concourse.bass_utils import run_bass_kernel_spmd

    nc = _get_nc()
    in_maps = _make_in_maps(**inputs)
    res = run_bass_kernel_spmd(nc, in_maps, core_ids=list(range(B)))
    return np.stack([res.results[b]["out"] for b in range(B)], axis=0)
